# revision 14
# baseline (speedup 1.0000x reference)
"""Trainium2 Bass kernel for nn_Aggregator (gnn_message_passing), v2.

pooled[B,D] = owner_masks.f32 @ ((nodes@Wt + bt) * sigmoid(nodes@Wg + bg))

Sharding: nodes (and owner_masks columns) split along N across 8 cores;
host sums the per-core [B, D] partials.

v2 design (vs v1's node-partition layout): mm1 runs TRANSPOSED
(stationary = W, psum_dT/gT are [D, nodes]) so both biases fold into
per-partition scalar operands:
  ACT : gatesT = sigmoid(psum_gT + bg)            (bias fused, 1 pass)
  DVE : msgT   = (psum_dT + bt) * gatesT          (scalar_tensor_tensor)
mm2 then needs msg back in [node, D]: PE transposes it per 128-tile
(fp16 -> fp16 PSUM) and ACT/DVE evict-copy pairs of chunks back to SBUF
(one wide op per 2 chunks amortizes per-op access-latency init; fp16
PSUM reads earn DVE's 2x_1p mode). Masks are staged in fp8e4 (0/1
exact; fp8-stationary x fp16-moving matmul verified on HW) halving mask
DMA. mm2 is D-wide only (no [B,2D] trick, no host bias fix-up).

Schedule: chunks of 512 nodes (last chunk is a single 128-node tile to
trim padding); per-index emission order is mm2(c-MD), transpose(pair at
c-TD), front(c) so every engine's in-order queue sees oldest-deps-first;
psum_dT/gT rotate through 5 single-bank PSUM slots, pair transposes
through 2 banks, the [B,D] accumulator holds the last bank. Slab input
DMAs prefetch one slab ahead on the SP ring. Cost-model (TimelineSim):
116.6us/core, engines ~86-90%% busy (PE 104.3us = 4 matmul rows/node
floor at fp16; DVE 105.1 = stt + 60%% of evicts; ACT 100.2 = sigmoid +
40%% of evicts; DMA 67.3). Rejected variants (all sim-verified worse):
DMA XBAR transposes in any mix (their ~2.5us round trip + DMA
instructions holding the issuing SEQ through sem waits starve the
pipeline, and any PE idling drops its pstate clock, inflating matmuls
2x); ACT evicts beyond ~40%% (delays the critical mm1_g->sigmoid->stt
chain); un-paired evicts; [B,2D] mm2 with host bias fix-up (v1);
node-partition mm1 with tensor-tensor bias adds (v1: elementwise floor
1081ns/chunk vs this layout's 800).
"""

import json

import numpy as np
import ml_dtypes

import concourse.bass as bass
import concourse.mybir as mybir
import concourse.tile as tile
from concourse import bass2jax as _b2j
from concourse import bass_utils as _bu
from concourse.bass_utils import run_bass_kernel_spmd


def _split_excess_waits_json(bir_json) -> bytes:
    """Walrus in this container accepts at most 1 embedded sem-wait per
    instruction (2 for EventSemaphore). Tile emits instructions (notably the
    kernel-tail Drain) with more. Move excess waits onto injected
    EventSemaphore instructions placed immediately before the offender in
    the same engine stream — identical blocking semantics."""
    if isinstance(bir_json, str):
        bir_json = bir_json.encode()
    d = json.loads(bir_json)
    counter = [0]

    def fix_block(b):
        new = []
        for inst in b.get("instructions", []):
            si = inst.get("sync_info")
            waits = (si or {}).get("on_wait") or []
            cap = 2 if inst.get("opcode") == "EventSemaphore" else 1
            if len(waits) > cap:
                keep, excess = waits[:cap], waits[cap:]
                for j in range(0, len(excess), 2):
                    counter[0] += 1
                    new.append(
                        {
                            "debug": inst.get("debug"),
                            "engine": inst["engine"],
                            "ins": [],
                            "outs": [],
                            "name": f"antsplit_ev_{counter[0]}",
                            "opcode": "EventSemaphore",
                            "sync_info": {
                                "on_update": [],
                                "on_wait": excess[j : j + 2],
                            },
                        }
                    )
                si["on_wait"] = keep
            new.append(inst)
        b["instructions"] = new
        for sb in b.get("blocks", []):
            fix_block(sb)

    for f in d.get("functions", []):
        for blk in f.get("blocks", []):
            fix_block(blk)
    return json.dumps(d).encode()


if not getattr(_bu, "_ant_split_waits_patched", False):
    _orig_compile_bir_kernel = _bu.compile_bir_kernel

    def _patched_compile_bir_kernel(bir_json, tmpdir, neff_name="file.neff"):
        return _orig_compile_bir_kernel(
            _split_excess_waits_json(bir_json), tmpdir, neff_name
        )

    _bu.compile_bir_kernel = _patched_compile_bir_kernel
    _b2j.compile_bir_kernel = _patched_compile_bir_kernel
    _bu._ant_split_waits_patched = True

N_CORES = 8
N_TOTAL = 500_000
B = 128
S = 128
D = 128
P = 128

N_PER_CORE = N_TOTAL // N_CORES          # 62500
TILES_PER_CHUNK = 4
CHUNK = TILES_PER_CHUNK * P              # 512
# slabs (in chunks): small first slabs so compute starts early. The final
# chunk is a 1-tile mini-chunk: 122 full chunks + 1 tile = 62592 staged
# nodes, trimming ~1.9 chunks of dead padding work vs rounding up to 124.
SLAB_CHUNKS = [4, 4, 4] + [8] * 13 + [7]
N_CHUNKS = sum(SLAB_CHUNKS)              # 123 (last chunk is 1 tile)
TPC_OF = [TILES_PER_CHUNK] * (N_CHUNKS - 1) + [1]   # tiles per chunk
TILE_OFF = [0]
for _t in TPC_OF:
    TILE_OFF.append(TILE_OFF[-1] + _t)
N_TILES = TILE_OFF[-1]                   # 489
N_PAD = N_TILES * P                      # 62592

F16 = mybir.dt.float16
F32 = mybir.dt.float32
F8 = mybir.dt.float8e4
NP_F16 = np.float16
NP_F8 = ml_dtypes.float8_e4m3

OPTS = {
    # PE-pair evicts whose index mod 5 is in this set run on ACT, the rest
    # on DVE (empty = all DVE). ~40% ACT balances ACT (sigmoid-laden)
    # against DVE (stt-laden).
    "act_evict_phases": (1, 3),
    "warm_mms": 36,
    "masks_fp8": True,
    "touches": True,
    # software pipelining: in PE program order, the transpose of pair p is
    # emitted TD chunks after p's second chunk, and chunk c's mm2 MD chunks
    # after c, so their cross-engine deps (ACT sigmoid -> DVE stt ->
    # [transpose+evict | DMA transpose]) are resolved before PE's in-order
    # queue reaches them.
    "td": 3,
    "md": 10,
}


def build_bass() -> bass.Bass:
    nc = bass.Bass()

    nodesT = nc.dram_tensor("nodesT", [P, N_PAD], F16, kind="ExternalInput").ap()
    mk_dt = F8 if OPTS["masks_fp8"] else F16
    masksT = nc.dram_tensor(
        "masksT", [P, N_TILES, B], mk_dt, kind="ExternalInput"
    ).ap()
    # packed constants: one fp16 [S, 3D] = [Wg | Wt | I], one fp32 [P, 2] =
    # [bt | bg] — two HWDGE dispatches instead of six at startup
    cst16_d = nc.dram_tensor("cst16", [S, 3 * D], F16, kind="ExternalInput").ap()
    cst32_d = nc.dram_tensor("cst32", [P, 2], F32, kind="ExternalInput").ap()
    out_d = nc.dram_tensor("out", [B, D], F32, kind="ExternalOutput").ap()

    with tile.TileContext(nc) as tc:
        with (
            tc.tile_pool(name="consts", bufs=1) as consts,
            tc.tile_pool(name="scratch", bufs=1) as scratch,
            tc.tile_pool(name="nodes", bufs=3) as nodes_pool,
            tc.tile_pool(name="masks", bufs=3) as masks_pool,
            tc.tile_pool(name="gT", bufs=3) as g_pool,
            tc.tile_pool(name="msgT", bufs=10) as mt_pool,
            tc.tile_pool(name="msg", bufs=14) as m_pool,
            tc.tile_pool(name="outs", bufs=1) as out_pool,
            tc.tile_pool(name="ps", bufs=5, space="PSUM") as ps_pool,
            tc.tile_pool(name="pm", bufs=2, space="PSUM") as pm_pool,
            tc.tile_pool(name="acc", bufs=1, space="PSUM") as acc_pool,
        ):
            MAX_SLAB = max(SLAB_CHUNKS)

            def emit_slab(first_chunk, s_chunks):
                to = TILE_OFF[first_chunk]
                n_t = TILE_OFF[first_chunk + s_chunks] - to
                slab_n = n_t * P
                slab_off = to * P
                nod_slab = nodes_pool.tile([P, MAX_SLAB * CHUNK], F16, tag="nod")
                nc.sync.dma_start(
                    nod_slab[:, :slab_n],
                    nodesT[:, slab_off : slab_off + slab_n],
                )
                mk_slab = masks_pool.tile(
                    [P, MAX_SLAB * TILES_PER_CHUNK, B], mk_dt, tag="mk"
                )
                nc.sync.dma_start(
                    mk_slab[:, :n_t, :],
                    masksT[:, to : to + n_t, :],
                )
                return nod_slab, mk_slab

            cst16 = consts.tile([S, 3 * D], F16)
            nc.sync.dma_start(cst16[:], cst16_d)
            # first node slab right after the consts: its (large) transfer
            # overlaps the scalar-const load and the warm-up matmuls
            slab_n0 = SLAB_CHUNKS[0] * CHUNK
            nod_slab0 = nodes_pool.tile([P, MAX_SLAB * CHUNK], F16, tag="nod")
            nc.sync.dma_start(nod_slab0[:, :slab_n0], nodesT[:, :slab_n0])
            cst32 = consts.tile([P, 2], F32)
            nc.sync.dma_start(cst32[:], cst32_d)
            wg_sb = cst16[:, 0:D]
            wt_sb = cst16[:, D : 2 * D]
            id_sb = cst16[:, 2 * D : 3 * D]
            bt_sb = cst32[:, 0:1]
            bg_sb = cst32[:, 1:2]

            # Warm-up: the PE pstate clock needs ~3us of continuous busy to
            # reach full speed, but the real weights arrive only at ~3.4us
            # (cst16 DMA). Memset a scratch operand via the otherwise-idle
            # Pool engine (~1us, no DMA dependency) and run the warm matmuls
            # on it, so PE is busy from ~1.2us through the first real mm1
            # and enters the hot loop fully ramped.
            if OPTS["warm_mms"]:
                warm_src = scratch.tile([P, D], F16)
                nc.gpsimd.memset(warm_src[:], 0.0)
                warm_ps = ps_pool.tile([P, CHUNK], F32, tag="ps")
                for _ in range(OPTS["warm_mms"]):
                    nc.tensor.matmul(
                        warm_ps[:, :D], warm_src[:], warm_src[:],
                        start=True, stop=True,
                    )
            # One-time const touches: absorb the const-DMA semaphores into
            # each engine's observed clock so hot-loop instructions never
            # need a second (DMA) wait slot.
            if OPTS["touches"]:
                dve_scratch = scratch.tile([P, 4], F32)
                nc.vector.tensor_copy(out=dve_scratch[:, :1], in_=bt_sb)
                nc.scalar.copy(out=dve_scratch[:1, 2:3], in_=dve_scratch[:1, :1])
                nc.tensor.ldweights(wt_sb[:, :1])
            mk_slab0 = masks_pool.tile(
                [P, MAX_SLAB * TILES_PER_CHUNK, B], mk_dt, tag="mk"
            )
            nc.sync.dma_start(
                mk_slab0[:, : SLAB_CHUNKS[0] * TILES_PER_CHUNK, :],
                masksT[:, : SLAB_CHUNKS[0] * TILES_PER_CHUNK, :],
            )

            pooled = acc_pool.tile([B, D], F32)

            TPC = TILES_PER_CHUNK
            TD, MD = OPTS["td"], OPTS["md"]

            # per-chunk state kept alive between pipeline stages
            state = {}          # c -> dict(mk=..., pair=...)
            pair_state = {}     # pair idx -> dict(msgT=..., msg=...)
            n_pe_evict = [0]
            n_mm2 = [0]

            # slab schedule keyed by the chunk index ONE SLAB AHEAD: slab
            # s+1's DMAs are emitted when slab s's first chunk is processed,
            # so input transfers always overlap the previous slab's compute.
            slab_info = []
            acc_c = 0
            for s, s_chunks in enumerate(SLAB_CHUNKS):
                slab_info.append((acc_c, s_chunks))
                acc_c += s_chunks
            prefetch_at = {}     # front chunk idx -> slab idx to emit
            acc_c = 0
            for s, s_chunks in enumerate(SLAB_CHUNKS):
                if s + 1 < len(SLAB_CHUNKS):
                    prefetch_at[acc_c] = s + 1
                acc_c += s_chunks
            chunk_slab = []      # chunk idx -> (slab idx, chunk-within-slab)
            for s, s_chunks in enumerate(SLAB_CHUNKS):
                for cs in range(s_chunks):
                    chunk_slab.append((s, cs))
            slabs = {0: (nod_slab0, mk_slab0)}

            def emit_front(c):
                if c in prefetch_at:
                    s = prefetch_at[c]
                    slabs[s] = emit_slab(*slab_info[s])
                s, cs = chunk_slab[c]
                nod_slab, mk_slab = slabs[s]
                tpc = TPC_OF[c]
                n = tpc * P
                wo = TILE_OFF[c] - TILE_OFF[slab_info[s][0]]
                nod = nod_slab[:, wo * P : wo * P + n]
                mk = mk_slab[:, wo : wo + tpc, :]

                # gate path first: it heads the longest cross-engine chain
                # (mm1_g -> sigmoid -> stt), and psum_dT's WAR slot-recycle
                # (freed by stt) gains slack from mm1_d running second
                psum_gT = ps_pool.tile([P, CHUNK], F32, tag="ps")
                psum_dT = ps_pool.tile([P, CHUNK], F32, tag="ps")
                nc.tensor.matmul(
                    psum_gT[:, :n], wg_sb, nod, start=True, stop=True
                )
                nc.tensor.matmul(
                    psum_dT[:, :n], wt_sb, nod, start=True, stop=True
                )

                gT = g_pool.tile([P, CHUNK], F16, tag="g")
                nc.scalar.activation(
                    gT[:, :n],
                    psum_gT[:, :n],
                    mybir.ActivationFunctionType.Sigmoid,
                    bias=bg_sb,
                    scale=1.0,
                )
                # msgT lives in per-PAIR tiles so the transpose+evict runs
                # once per pair: half the per-op overhead
                pi, half = divmod(c, 2)
                if half == 0:
                    msgT = mt_pool.tile([P, 2 * CHUNK], F16, tag="mt", name="msgT")
                    pair_state[pi] = {"msgT": msgT}
                msgT = pair_state[pi]["msgT"]
                nc.vector.scalar_tensor_tensor(
                    out=msgT[:, half * CHUNK : half * CHUNK + n],
                    in0=psum_dT[:, :n],
                    scalar=bt_sb,
                    in1=gT[:, :n],
                    op0=mybir.AluOpType.add,
                    op1=mybir.AluOpType.mult,
                )
                state[c] = {"mk": mk, "pair": pi}

            def pair_tiles(p):
                # (msgT-slot, msg-slot) index pairs for the pair's live tiles
                out = []
                for half in (0, 1):
                    ch = 2 * p + half
                    if ch < N_CHUNKS:
                        for t in range(TPC_OF[ch]):
                            out.append((half * TPC + t, half * TPC + t))
                return out

            def emit_transpose(p):
                st = pair_state[p]
                msgT = st["msgT"]
                msg = m_pool.tile([P, 2 * TPC, P], F16, tag="m")
                tiles = pair_tiles(p)
                psum_m = pm_pool.tile([P, 2 * TPC, P], F16, tag="pm")
                for src, dst in tiles:
                    nc.tensor.transpose(
                        psum_m[:, dst, :],
                        msgT[:, src * P : (src + 1) * P],
                        id_sb,
                    )
                nt = tiles[-1][1] + 1
                if n_pe_evict[0] % 5 in OPTS["act_evict_phases"]:
                    nc.scalar.copy(out=msg[:, :nt, :], in_=psum_m[:, :nt, :])
                else:
                    nc.vector.tensor_copy(
                        out=msg[:, :nt, :], in_=psum_m[:, :nt, :]
                    )
                n_pe_evict[0] += 1
                st["msg"] = msg

            def emit_mm2(c):
                st = state.pop(c)
                pi, half = divmod(c, 2)
                pst = pair_state[pi]
                msg = pst["msg"]
                for t in range(TPC_OF[c]):
                    n_mm2[0] += 1
                    nc.tensor.matmul(
                        pooled[:],
                        st["mk"][:, t, :],
                        msg[:, half * TPC + t, :],
                        start=(n_mm2[0] == 1),
                        stop=(n_mm2[0] == N_TILES),
                        skip_group_check=True,
                    )
                if half == 1 or c == N_CHUNKS - 1:
                    del pair_state[pi]

            # stage order per index: oldest work first, so each engine's
            # in-order queue never has a young wait blocking old ready work.
            # The pair transpose runs TD chunks after the pair's second
            # chunk; each chunk's mm2 runs MD chunks after its front stage —
            # PSUM accumulation order is irrelevant, so start/stop follow
            # emission order via the n_mm2 counter.
            mm2_at = {}
            for c in range(N_CHUNKS):
                dl = c + MD
                # tail clamp: compress the post-loop mm2 backlog, but never
                # ahead of the pair's transpose emission (PE is in-order —
                # an mm2 enqueued before the transposes feeding it deadlocks)
                trans_idx = min((c // 2) * 2 + 1, N_CHUNKS - 1) + TD
                dl = max(trans_idx + 1, min(dl, N_CHUNKS + 2))
                mm2_at.setdefault(dl, []).append(c)
            for cc in range(N_CHUNKS + MD + 3):
                for c in mm2_at.get(cc, ()):
                    emit_mm2(c)
                tc_c = cc - TD          # a pair's last chunk at stage TD
                if TD <= cc and tc_c < N_CHUNKS and (
                    tc_c % 2 == 1 or tc_c == N_CHUNKS - 1
                ):
                    emit_transpose(tc_c // 2)
                if cc < N_CHUNKS:
                    emit_front(cc)

            res = out_pool.tile([B, D], F32)
            nc.vector.tensor_copy(out=res[:], in_=pooled[:])
            nc.sync.dma_start(out_d, res[:])

    return nc


_CACHE: dict = {}


def _get_bass() -> bass.Bass:
    if "nc" not in _CACHE:
        _CACHE["nc"] = build_bass()
    return _CACHE["nc"]


def _prepare_in_maps(nodes, owner_masks, Wt, bt, Wg, bg):
    nodes_h = np.asarray(nodes, dtype=NP_F16)
    masks = np.asarray(owner_masks)
    np_mk = NP_F8 if OPTS["masks_fp8"] else NP_F16
    cst16 = np.concatenate(
        [
            np.asarray(Wg, dtype=NP_F16),
            np.asarray(Wt, dtype=NP_F16),
            np.eye(P, dtype=NP_F16),
        ],
        axis=1,
    )
    cst32 = np.stack(
        [np.asarray(bt, np.float32), np.asarray(bg, np.float32)], axis=1
    )
    cst16 = np.ascontiguousarray(cst16)
    cst32 = np.ascontiguousarray(cst32)

    in_maps = []
    for core in range(N_CORES):
        off = core * N_PER_CORE
        ncr = np.zeros((P, N_PAD), dtype=NP_F16)
        ncr[:, :N_PER_CORE] = nodes_h[off : off + N_PER_CORE].T
        mp = np.zeros((B, N_PAD), dtype=np.int8)
        mp[:, :N_PER_CORE] = masks[:, off : off + N_PER_CORE]
        mkt = np.ascontiguousarray(
            mp.reshape(B, N_TILES, P).transpose(2, 1, 0)
        ).astype(np_mk)
        in_maps.append(
            {
                "nodesT": ncr,
                "masksT": mkt,
                "cst16": cst16,
                "cst32": cst32,
            }
        )
    return in_maps


def run(inputs: dict, trace: bool = False):
    """Run the kernel. Returns (pooled [B, D] float32, BassKernelResults)."""
    nc = _get_bass()
    in_maps = _prepare_in_maps(**inputs)
    rb = run_bass_kernel_spmd(
        nc, in_maps, core_ids=list(range(N_CORES)), trace=trace
    )
    parts = np.stack([r["out"].astype(np.float64) for r in rb.results])
    pooled = parts.sum(axis=0)
    return pooled.astype(np.float32), rb


def kernel(**inputs) -> np.ndarray:
    try:
        out, _ = run(inputs, trace=False)
    except Exception:
        # transient device errors (e.g. residual bad state from a previous
        # crashed NEFF) have been observed once; one retry clears them
        out, _ = run(inputs, trace=False)
    return out


if __name__ == "__main__":
    rng = np.random.default_rng(0)
    demo = {
        "nodes": rng.standard_normal((N_TOTAL, S), dtype=np.float32),
        "owner_masks": rng.integers(0, 2, (B, N_TOTAL)).astype(np.int32),
        "Wt": rng.standard_normal((S, D), dtype=np.float32) * 0.09,
        "bt": rng.standard_normal(D).astype(np.float32) * 0.09,
        "Wg": rng.standard_normal((S, D), dtype=np.float32) * 0.09,
        "bg": rng.standard_normal(D).astype(np.float32) * 0.09,
    }
    out = kernel(**demo)
    print(out.shape, out.dtype, np.abs(out).mean())


# revision 16
# speedup vs baseline: 1.0020x; 1.0020x over previous
"""Trainium2 Bass kernel for nn_Aggregator (gnn_message_passing), v2.

pooled[B,D] = owner_masks.f32 @ ((nodes@Wt + bt) * sigmoid(nodes@Wg + bg))

Sharding: nodes (and owner_masks columns) split along N across 8 cores;
host sums the per-core [B, D] partials.

v2 design (vs v1's node-partition layout): mm1 runs TRANSPOSED
(stationary = W, psum_dT/gT are [D, nodes]) so both biases fold into
per-partition scalar operands:
  ACT : gatesT = sigmoid(psum_gT + bg)            (bias fused, 1 pass)
  DVE : msgT   = (psum_dT + bt) * gatesT          (scalar_tensor_tensor)
mm2 then needs msg back in [node, D]: PE transposes it per 128-tile
(fp16 -> fp16 PSUM) and ACT/DVE evict-copy pairs of chunks back to SBUF
(one wide op per 2 chunks amortizes per-op access-latency init; fp16
PSUM reads earn DVE's 2x_1p mode). Masks are staged in fp8e4 (0/1
exact; fp8-stationary x fp16-moving matmul verified on HW) halving mask
DMA. mm2 is D-wide only (no [B,2D] trick, no host bias fix-up).

Schedule: chunks of 512 nodes (last chunk is a single 128-node tile to
trim padding); per-index emission order is mm2(c-MD), transpose(pair at
c-TD), front(c) so every engine's in-order queue sees oldest-deps-first;
psum_dT/gT rotate through 5 single-bank PSUM slots, pair transposes
through 2 banks, the [B,D] accumulator holds the last bank. Slab input
DMAs prefetch one slab ahead on the SP ring. Cost-model (TimelineSim):
116.3us/core, engines ~86-90%% busy (PE 104.3us = 4 matmul rows/node
floor at fp16; DVE 105.1 = stt + 60%% of evicts; ACT 100.2 = sigmoid +
40%% of evicts; DMA 67.3). Rejected variants (all sim-verified worse):
DMA XBAR transposes in any mix (their ~2.5us round trip + DMA
instructions holding the issuing SEQ through sem waits starve the
pipeline, and any PE idling drops its pstate clock, inflating matmuls
2x); ACT evicts beyond ~40%% (delays the critical mm1_g->sigmoid->stt
chain); un-paired evicts; [B,2D] mm2 with host bias fix-up (v1);
node-partition mm1 with tensor-tensor bias adds (v1: elementwise floor
1081ns/chunk vs this layout's 800).
"""

import json

import numpy as np
import ml_dtypes

import concourse.bass as bass
import concourse.mybir as mybir
import concourse.tile as tile
from concourse import bass2jax as _b2j
from concourse import bass_utils as _bu
from concourse.bass_utils import run_bass_kernel_spmd


def _split_excess_waits_json(bir_json) -> bytes:
    """Walrus in this container accepts at most 1 embedded sem-wait per
    instruction (2 for EventSemaphore). Tile emits instructions (notably the
    kernel-tail Drain) with more. Move excess waits onto injected
    EventSemaphore instructions placed immediately before the offender in
    the same engine stream — identical blocking semantics."""
    if isinstance(bir_json, str):
        bir_json = bir_json.encode()
    d = json.loads(bir_json)
    counter = [0]

    def fix_block(b):
        new = []
        for inst in b.get("instructions", []):
            si = inst.get("sync_info")
            waits = (si or {}).get("on_wait") or []
            cap = 2 if inst.get("opcode") == "EventSemaphore" else 1
            if len(waits) > cap:
                keep, excess = waits[:cap], waits[cap:]
                for j in range(0, len(excess), 2):
                    counter[0] += 1
                    new.append(
                        {
                            "debug": inst.get("debug"),
                            "engine": inst["engine"],
                            "ins": [],
                            "outs": [],
                            "name": f"antsplit_ev_{counter[0]}",
                            "opcode": "EventSemaphore",
                            "sync_info": {
                                "on_update": [],
                                "on_wait": excess[j : j + 2],
                            },
                        }
                    )
                si["on_wait"] = keep
            new.append(inst)
        b["instructions"] = new
        for sb in b.get("blocks", []):
            fix_block(sb)

    for f in d.get("functions", []):
        for blk in f.get("blocks", []):
            fix_block(blk)
    return json.dumps(d).encode()


if not getattr(_bu, "_ant_split_waits_patched", False):
    _orig_compile_bir_kernel = _bu.compile_bir_kernel

    def _patched_compile_bir_kernel(bir_json, tmpdir, neff_name="file.neff"):
        return _orig_compile_bir_kernel(
            _split_excess_waits_json(bir_json), tmpdir, neff_name
        )

    _bu.compile_bir_kernel = _patched_compile_bir_kernel
    _b2j.compile_bir_kernel = _patched_compile_bir_kernel
    _bu._ant_split_waits_patched = True

N_CORES = 8
N_TOTAL = 500_000
B = 128
S = 128
D = 128
P = 128

N_PER_CORE = N_TOTAL // N_CORES          # 62500
TILES_PER_CHUNK = 4
CHUNK = TILES_PER_CHUNK * P              # 512
# slabs (in chunks): small first slabs so compute starts early. The final
# chunk is a 1-tile mini-chunk: 122 full chunks + 1 tile = 62592 staged
# nodes, trimming ~1.9 chunks of dead padding work vs rounding up to 124.
SLAB_CHUNKS = [4, 4, 4] + [8] * 13 + [7]
N_CHUNKS = sum(SLAB_CHUNKS)              # 123 (last chunk is 1 tile)
TPC_OF = [TILES_PER_CHUNK] * (N_CHUNKS - 1) + [1]   # tiles per chunk
TILE_OFF = [0]
for _t in TPC_OF:
    TILE_OFF.append(TILE_OFF[-1] + _t)
N_TILES = TILE_OFF[-1]                   # 489
N_PAD = N_TILES * P                      # 62592

F16 = mybir.dt.float16
F32 = mybir.dt.float32
F8 = mybir.dt.float8e4
NP_F16 = np.float16
NP_F8 = ml_dtypes.float8_e4m3

OPTS = {
    # PE-pair evicts whose index mod 5 is in this set run on ACT, the rest
    # on DVE (empty = all DVE). ~40% ACT balances ACT (sigmoid-laden)
    # against DVE (stt-laden).
    "act_evict_phases": (1, 3),
    "warm_mms": 36,
    "masks_fp8": True,
    "touches": True,
    # software pipelining: in PE program order, the transpose of pair p is
    # emitted TD chunks after p's second chunk, and chunk c's mm2 MD chunks
    # after c, so their cross-engine deps (ACT sigmoid -> DVE stt ->
    # [transpose+evict | DMA transpose]) are resolved before PE's in-order
    # queue reaches them.
    "td": 3,
    "md": 10,
}


def build_bass() -> bass.Bass:
    nc = bass.Bass()

    nodesT = nc.dram_tensor("nodesT", [P, N_PAD], F16, kind="ExternalInput").ap()
    mk_dt = F8 if OPTS["masks_fp8"] else F16
    masksT = nc.dram_tensor(
        "masksT", [P, N_TILES, B], mk_dt, kind="ExternalInput"
    ).ap()
    # packed constants: one fp16 [S, 3D] = [Wg | Wt | I], one fp32 [P, 2] =
    # [bt | bg] — two HWDGE dispatches instead of six at startup
    cst16_d = nc.dram_tensor("cst16", [S, 3 * D], F16, kind="ExternalInput").ap()
    cst32_d = nc.dram_tensor("cst32", [P, 2], F32, kind="ExternalInput").ap()
    out_d = nc.dram_tensor("out", [B, D], F32, kind="ExternalOutput").ap()

    with tile.TileContext(nc) as tc:
        with (
            tc.tile_pool(name="consts", bufs=1) as consts,
            tc.tile_pool(name="scratch", bufs=1) as scratch,
            tc.tile_pool(name="nodes", bufs=3) as nodes_pool,
            tc.tile_pool(name="masks", bufs=3) as masks_pool,
            tc.tile_pool(name="gT", bufs=4) as g_pool,
            tc.tile_pool(name="msgT", bufs=10) as mt_pool,
            tc.tile_pool(name="msg", bufs=14) as m_pool,
            tc.tile_pool(name="outs", bufs=1) as out_pool,
            tc.tile_pool(name="ps", bufs=5, space="PSUM") as ps_pool,
            tc.tile_pool(name="pm", bufs=2, space="PSUM") as pm_pool,
            tc.tile_pool(name="acc", bufs=1, space="PSUM") as acc_pool,
        ):
            MAX_SLAB = max(SLAB_CHUNKS)

            def emit_slab(first_chunk, s_chunks):
                to = TILE_OFF[first_chunk]
                n_t = TILE_OFF[first_chunk + s_chunks] - to
                slab_n = n_t * P
                slab_off = to * P
                nod_slab = nodes_pool.tile([P, MAX_SLAB * CHUNK], F16, tag="nod")
                nc.sync.dma_start(
                    nod_slab[:, :slab_n],
                    nodesT[:, slab_off : slab_off + slab_n],
                )
                mk_slab = masks_pool.tile(
                    [P, MAX_SLAB * TILES_PER_CHUNK, B], mk_dt, tag="mk"
                )
                nc.sync.dma_start(
                    mk_slab[:, :n_t, :],
                    masksT[:, to : to + n_t, :],
                )
                return nod_slab, mk_slab

            cst16 = consts.tile([S, 3 * D], F16)
            nc.sync.dma_start(cst16[:], cst16_d)
            # first node slab right after the consts: its (large) transfer
            # overlaps the scalar-const load and the warm-up matmuls
            slab_n0 = SLAB_CHUNKS[0] * CHUNK
            nod_slab0 = nodes_pool.tile([P, MAX_SLAB * CHUNK], F16, tag="nod")
            nc.sync.dma_start(nod_slab0[:, :slab_n0], nodesT[:, :slab_n0])
            cst32 = consts.tile([P, 2], F32)
            nc.sync.dma_start(cst32[:], cst32_d)
            wg_sb = cst16[:, 0:D]
            wt_sb = cst16[:, D : 2 * D]
            id_sb = cst16[:, 2 * D : 3 * D]
            bt_sb = cst32[:, 0:1]
            bg_sb = cst32[:, 1:2]

            # Warm-up: the PE pstate clock needs ~3us of continuous busy to
            # reach full speed, but the real weights arrive only at ~3.4us
            # (cst16 DMA). Memset a scratch operand via the otherwise-idle
            # Pool engine (~1us, no DMA dependency) and run the warm matmuls
            # on it, so PE is busy from ~1.2us through the first real mm1
            # and enters the hot loop fully ramped.
            if OPTS["warm_mms"]:
                warm_src = scratch.tile([P, D], F16)
                nc.gpsimd.memset(warm_src[:], 0.0)
                warm_ps = ps_pool.tile([P, CHUNK], F32, tag="ps")
                for _ in range(OPTS["warm_mms"]):
                    nc.tensor.matmul(
                        warm_ps[:, :D], warm_src[:], warm_src[:],
                        start=True, stop=True,
                    )
            # One-time const touches: absorb the const-DMA semaphores into
            # each engine's observed clock so hot-loop instructions never
            # need a second (DMA) wait slot.
            if OPTS["touches"]:
                dve_scratch = scratch.tile([P, 4], F32)
                nc.vector.tensor_copy(out=dve_scratch[:, :1], in_=bt_sb)
                nc.scalar.copy(out=dve_scratch[:1, 2:3], in_=dve_scratch[:1, :1])
                nc.tensor.ldweights(wt_sb[:, :1])
            mk_slab0 = masks_pool.tile(
                [P, MAX_SLAB * TILES_PER_CHUNK, B], mk_dt, tag="mk"
            )
            nc.sync.dma_start(
                mk_slab0[:, : SLAB_CHUNKS[0] * TILES_PER_CHUNK, :],
                masksT[:, : SLAB_CHUNKS[0] * TILES_PER_CHUNK, :],
            )

            pooled = acc_pool.tile([B, D], F32)

            TPC = TILES_PER_CHUNK
            TD, MD = OPTS["td"], OPTS["md"]

            # per-chunk state kept alive between pipeline stages
            state = {}          # c -> dict(mk=..., pair=...)
            pair_state = {}     # pair idx -> dict(msgT=..., msg=...)
            n_pe_evict = [0]
            n_mm2 = [0]

            # slab schedule keyed by the chunk index ONE SLAB AHEAD: slab
            # s+1's DMAs are emitted when slab s's first chunk is processed,
            # so input transfers always overlap the previous slab's compute.
            slab_info = []
            acc_c = 0
            for s, s_chunks in enumerate(SLAB_CHUNKS):
                slab_info.append((acc_c, s_chunks))
                acc_c += s_chunks
            prefetch_at = {}     # front chunk idx -> slab idx to emit
            acc_c = 0
            for s, s_chunks in enumerate(SLAB_CHUNKS):
                if s + 1 < len(SLAB_CHUNKS):
                    prefetch_at[acc_c] = s + 1
                acc_c += s_chunks
            chunk_slab = []      # chunk idx -> (slab idx, chunk-within-slab)
            for s, s_chunks in enumerate(SLAB_CHUNKS):
                for cs in range(s_chunks):
                    chunk_slab.append((s, cs))
            slabs = {0: (nod_slab0, mk_slab0)}

            def emit_front(c):
                if c in prefetch_at:
                    s = prefetch_at[c]
                    slabs[s] = emit_slab(*slab_info[s])
                s, cs = chunk_slab[c]
                nod_slab, mk_slab = slabs[s]
                tpc = TPC_OF[c]
                n = tpc * P
                wo = TILE_OFF[c] - TILE_OFF[slab_info[s][0]]
                nod = nod_slab[:, wo * P : wo * P + n]
                mk = mk_slab[:, wo : wo + tpc, :]

                # gate path first: it heads the longest cross-engine chain
                # (mm1_g -> sigmoid -> stt), and psum_dT's WAR slot-recycle
                # (freed by stt) gains slack from mm1_d running second
                psum_gT = ps_pool.tile([P, CHUNK], F32, tag="ps")
                psum_dT = ps_pool.tile([P, CHUNK], F32, tag="ps")
                nc.tensor.matmul(
                    psum_gT[:, :n], wg_sb, nod, start=True, stop=True
                )
                nc.tensor.matmul(
                    psum_dT[:, :n], wt_sb, nod, start=True, stop=True
                )

                gT = g_pool.tile([P, CHUNK], F16, tag="g")
                nc.scalar.activation(
                    gT[:, :n],
                    psum_gT[:, :n],
                    mybir.ActivationFunctionType.Sigmoid,
                    bias=bg_sb,
                    scale=1.0,
                )
                # msgT lives in per-PAIR tiles so the transpose+evict runs
                # once per pair: half the per-op overhead
                pi, half = divmod(c, 2)
                if half == 0:
                    msgT = mt_pool.tile([P, 2 * CHUNK], F16, tag="mt", name="msgT")
                    pair_state[pi] = {"msgT": msgT}
                msgT = pair_state[pi]["msgT"]
                nc.vector.scalar_tensor_tensor(
                    out=msgT[:, half * CHUNK : half * CHUNK + n],
                    in0=psum_dT[:, :n],
                    scalar=bt_sb,
                    in1=gT[:, :n],
                    op0=mybir.AluOpType.add,
                    op1=mybir.AluOpType.mult,
                )
                state[c] = {"mk": mk, "pair": pi}

            def pair_tiles(p):
                # (msgT-slot, msg-slot) index pairs for the pair's live tiles
                out = []
                for half in (0, 1):
                    ch = 2 * p + half
                    if ch < N_CHUNKS:
                        for t in range(TPC_OF[ch]):
                            out.append((half * TPC + t, half * TPC + t))
                return out

            def emit_transpose(p):
                st = pair_state[p]
                msgT = st["msgT"]
                msg = m_pool.tile([P, 2 * TPC, P], F16, tag="m")
                tiles = pair_tiles(p)
                psum_m = pm_pool.tile([P, 2 * TPC, P], F16, tag="pm")
                for src, dst in tiles:
                    nc.tensor.transpose(
                        psum_m[:, dst, :],
                        msgT[:, src * P : (src + 1) * P],
                        id_sb,
                    )
                nt = tiles[-1][1] + 1
                if n_pe_evict[0] % 5 in OPTS["act_evict_phases"]:
                    nc.scalar.copy(out=msg[:, :nt, :], in_=psum_m[:, :nt, :])
                else:
                    nc.vector.tensor_copy(
                        out=msg[:, :nt, :], in_=psum_m[:, :nt, :]
                    )
                n_pe_evict[0] += 1
                st["msg"] = msg

            def emit_mm2(c):
                st = state.pop(c)
                pi, half = divmod(c, 2)
                pst = pair_state[pi]
                msg = pst["msg"]
                for t in range(TPC_OF[c]):
                    n_mm2[0] += 1
                    nc.tensor.matmul(
                        pooled[:],
                        st["mk"][:, t, :],
                        msg[:, half * TPC + t, :],
                        start=(n_mm2[0] == 1),
                        stop=(n_mm2[0] == N_TILES),
                        skip_group_check=True,
                    )
                if half == 1 or c == N_CHUNKS - 1:
                    del pair_state[pi]

            # stage order per index: oldest work first, so each engine's
            # in-order queue never has a young wait blocking old ready work.
            # The pair transpose runs TD chunks after the pair's second
            # chunk; each chunk's mm2 runs MD chunks after its front stage —
            # PSUM accumulation order is irrelevant, so start/stop follow
            # emission order via the n_mm2 counter.
            mm2_at = {}
            for c in range(N_CHUNKS):
                dl = c + MD
                # tail clamp: compress the post-loop mm2 backlog, but never
                # ahead of the pair's transpose emission (PE is in-order —
                # an mm2 enqueued before the transposes feeding it deadlocks)
                trans_idx = min((c // 2) * 2 + 1, N_CHUNKS - 1) + TD
                dl = max(trans_idx + 1, min(dl, N_CHUNKS + 2))
                mm2_at.setdefault(dl, []).append(c)
            for cc in range(N_CHUNKS + MD + 3):
                for c in mm2_at.get(cc, ()):
                    emit_mm2(c)
                tc_c = cc - TD          # a pair's last chunk at stage TD
                if TD <= cc and tc_c < N_CHUNKS and (
                    tc_c % 2 == 1 or tc_c == N_CHUNKS - 1
                ):
                    emit_transpose(tc_c // 2)
                if cc < N_CHUNKS:
                    emit_front(cc)

            res = out_pool.tile([B, D], F32)
            nc.vector.tensor_copy(out=res[:], in_=pooled[:])
            nc.sync.dma_start(out_d, res[:])

    return nc


_CACHE: dict = {}


def _get_bass() -> bass.Bass:
    if "nc" not in _CACHE:
        _CACHE["nc"] = build_bass()
    return _CACHE["nc"]


def _prepare_in_maps(nodes, owner_masks, Wt, bt, Wg, bg):
    nodes_h = np.asarray(nodes, dtype=NP_F16)
    masks = np.asarray(owner_masks)
    np_mk = NP_F8 if OPTS["masks_fp8"] else NP_F16
    cst16 = np.concatenate(
        [
            np.asarray(Wg, dtype=NP_F16),
            np.asarray(Wt, dtype=NP_F16),
            np.eye(P, dtype=NP_F16),
        ],
        axis=1,
    )
    cst32 = np.stack(
        [np.asarray(bt, np.float32), np.asarray(bg, np.float32)], axis=1
    )
    cst16 = np.ascontiguousarray(cst16)
    cst32 = np.ascontiguousarray(cst32)

    in_maps = []
    for core in range(N_CORES):
        off = core * N_PER_CORE
        ncr = np.zeros((P, N_PAD), dtype=NP_F16)
        ncr[:, :N_PER_CORE] = nodes_h[off : off + N_PER_CORE].T
        mp = np.zeros((B, N_PAD), dtype=np.int8)
        mp[:, :N_PER_CORE] = masks[:, off : off + N_PER_CORE]
        mkt = np.ascontiguousarray(
            mp.reshape(B, N_TILES, P).transpose(2, 1, 0)
        ).astype(np_mk)
        in_maps.append(
            {
                "nodesT": ncr,
                "masksT": mkt,
                "cst16": cst16,
                "cst32": cst32,
            }
        )
    return in_maps


def run(inputs: dict, trace: bool = False):
    """Run the kernel. Returns (pooled [B, D] float32, BassKernelResults)."""
    nc = _get_bass()
    in_maps = _prepare_in_maps(**inputs)
    rb = run_bass_kernel_spmd(
        nc, in_maps, core_ids=list(range(N_CORES)), trace=trace
    )
    parts = np.stack([r["out"].astype(np.float64) for r in rb.results])
    pooled = parts.sum(axis=0)
    return pooled.astype(np.float32), rb


def kernel(**inputs) -> np.ndarray:
    try:
        out, _ = run(inputs, trace=False)
    except Exception:
        # transient device errors (e.g. residual bad state from a previous
        # crashed NEFF) have been observed once; one retry clears them
        out, _ = run(inputs, trace=False)
    return out


if __name__ == "__main__":
    rng = np.random.default_rng(0)
    demo = {
        "nodes": rng.standard_normal((N_TOTAL, S), dtype=np.float32),
        "owner_masks": rng.integers(0, 2, (B, N_TOTAL)).astype(np.int32),
        "Wt": rng.standard_normal((S, D), dtype=np.float32) * 0.09,
        "bt": rng.standard_normal(D).astype(np.float32) * 0.09,
        "Wg": rng.standard_normal((S, D), dtype=np.float32) * 0.09,
        "bg": rng.standard_normal(D).astype(np.float32) * 0.09,
    }
    out = kernel(**demo)
    print(out.shape, out.dtype, np.abs(out).mean())
